# revision 32
# baseline (speedup 1.0000x reference)
"""Causal multi-head attention forward on 8 Trainium2 NeuronCores.

Problem: nn_CoreAttention (SQ=SK=2048, B=2, NP=16 heads, HN=128, fp32).

Sharding: 32 (batch, head) pairs split 4 per core (tensor-parallel over
heads, data-parallel over batch). No collectives.

Per (b, n) pair, in transposed score orientation (sk on partitions):
    scoresT[sk, sq] = (K Q^T)                 (PE matmul, fp16 in, hn contracted)
    expT = exp(scoresT * 1/sqrt(HN) + mask)   (ScalarE, fp16 out)
    ctx_aug[sq, hn+1] = expT^T @ [V | 1]      (PE matmul, sk contracted;
                                               col hn holds the softmax denom)
    ctx = ctx_aug[:, :hn] * 1/ctx_aug[:, hn]  (DVE reciprocal + scale)

v2 structure (vs the v1 baseline):
  - q/k cast to fp16 on host; no on-device casts; FWL stays enabled.
  - sq chunks of 256 with the two 128-tiles SWAPPED (host pre-swap), so the
    diagonal j-tile's masked half is the trailing half of the chunk: the
    diagonal QK matmul streams only 128 valid cols and exp skips the rest.
  - the two triangular tiles per chunk land on 256 contiguous score cols:
    one DVE add with a single [128,256] additive tile handles all masking.
  - PV is software-pipelined one chunk behind QK so the PE never stalls on
    the exp latency of the chunk it just produced.
  - slot 0 loads q/k/v in small pieces (fast start); the last slot walks
    chunks descending so the kernel tail is the smallest chunk.
  - outputs accumulate in SBUF and leave as 4 wide DMAs per slot.
"""

import math
import numpy as np
from contextlib import ExitStack

import concourse.bacc as bacc
import concourse.tile as tile
from concourse import mybir

SQ, SK, B, NP, HN = 2048, 2048, 2, 16, 128
N_CORES = 8
SLOTS_PER_CORE = 4
P = 128
CHUNK = 256
N_CHUNKS = SQ // CHUNK      # 8
N_SK_TILES = SK // P        # 16
NEG = -60000.0

import os
GROUP = int(os.environ.get("ATT_GROUP", "6"))          # j-tiles per PSUM group
SC_BUFS = int(os.environ.get("ATT_SC_BUFS", "2"))
CX_BUFS = int(os.environ.get("ATT_CX_BUFS", "2"))
E_BUFS = int(os.environ.get("ATT_E_BUFS", "8"))

F32 = mybir.dt.float32
F16 = mybir.dt.float16


def _build_program():
    nc = bacc.Bacc()

    qT_d = nc.declare_dram_parameter("qT", [SLOTS_PER_CORE, P, SQ], F16, isOutput=False)
    kT_d = nc.declare_dram_parameter("kT", [SLOTS_PER_CORE, P, SK], F16, isOutput=False)
    v_d = nc.declare_dram_parameter(
        "v_aug", [SLOTS_PER_CORE, P, N_SK_TILES * (HN + 1)], F16, isOutput=False
    )
    # triT[p, c] = NEG if p < c else 0; ident2 = [I | I]
    triT_d = nc.declare_dram_parameter("triT", [P, P], F16, isOutput=False)
    id2_d = nc.declare_dram_parameter("ident2", [P, 2 * P], F16, isOutput=False)
    out_d = nc.declare_dram_parameter(
        "out", [SLOTS_PER_CORE, 4, P, 4 * HN], F32, isOutput=True
    )

    inv_norm = 1.0 / math.sqrt(HN)

    with tile.TileContext(nc) as tc, ExitStack() as ctx:
        qk_pool = ctx.enter_context(tc.tile_pool(name="qk", bufs=2))
        v_pool = ctx.enter_context(tc.tile_pool(name="v", bufs=2))
        m_pool = ctx.enter_context(tc.tile_pool(name="m", bufs=1))
        e_pool = ctx.enter_context(tc.tile_pool(name="e", bufs=E_BUFS))
        o_pool = ctx.enter_context(tc.tile_pool(name="o", bufs=4))
        r_pool = ctx.enter_context(tc.tile_pool(name="r", bufs=4))
        sc_ps = ctx.enter_context(tc.tile_pool(name="sc", bufs=SC_BUFS, space="PSUM"))
        cx_ps = ctx.enter_context(tc.tile_pool(name="cx", bufs=CX_BUFS, space="PSUM"))

        triT_sb = m_pool.tile([P, P], F16, tag="triT")
        nc.scalar.dma_start(triT_sb[:], triT_d[:])
        id2_sb = m_pool.tile([P, 2 * P], F16, tag="id2")

        # touch Exp immediately so the ACT table loads during the initial DMAs
        warm_in = m_pool.tile([P, 1], F32, tag="warm_in")
        nc.vector.memset(warm_in[:], 0.0)
        warm_out = m_pool.tile([P, 1], F32, tag="warm_out")
        nc.scalar.activation(
            warm_out[:], warm_in[:], mybir.ActivationFunctionType.Exp
        )



        # ---- per-slot input loading -------------------------------------
        def load_slot(slot):
            """Returns (kslice, qchunk, vslice) accessor fns for this slot."""
            if slot == 0:
                # pieces, issued in first-use order (chunks run descending)
                # across both HWDGE queues (sync + scalar run in parallel and
                # each queue moves ~55 GB/s, so split k into 4 transfers)
                kts = [
                    qk_pool.tile([P, 4 * P], F16, tag=f"k{pc}", name=f"k{pc}")
                    for pc in range(4)
                ]
                qts = [
                    qk_pool.tile([P, CHUNK], F16, tag=f"q{pc}", name=f"q{pc}")
                    for pc in range(N_CHUNKS)
                ]
                vts = [
                    v_pool.tile([P, 8 * (HN + 1)], F16, tag=f"v{pc}", name=f"v{pc}")
                    for pc in range(2)
                ]
                nc.sync.dma_start(kts[0][:], kT_d[slot][:, 0:512])
                nc.scalar.dma_start(qts[7][:], qT_d[slot][:, 7 * CHUNK : 8 * CHUNK])
                nc.sync.dma_start(kts[1][:], kT_d[slot][:, 512:1024])
                nc.scalar.dma_start(kts[2][:], kT_d[slot][:, 1024:1536])
                nc.sync.dma_start(id2_sb[:], id2_d[:])
                nc.scalar.dma_start(kts[3][:], kT_d[slot][:, 1536:2048])
                nc.sync.dma_start(qts[6][:], qT_d[slot][:, 6 * CHUNK : 7 * CHUNK])
                for pc in range(2):
                    nc.scalar.dma_start(
                        vts[pc][:],
                        v_d[slot][:, pc * 8 * (HN + 1) : (pc + 1) * 8 * (HN + 1)],
                    )
                for pc in range(N_CHUNKS - 3, -1, -1):
                    nc.sync.dma_start(
                        qts[pc][:], qT_d[slot][:, pc * CHUNK : (pc + 1) * CHUNK]
                    )
                kslice = lambda j: kts[j // 4][:, (j % 4) * P : (j % 4 + 1) * P]
                qchunk = lambda ci: qts[ci][:]
                vslice = lambda j: vts[j // 8][
                    :, (j % 8) * (HN + 1) : (j % 8 + 1) * (HN + 1)
                ]
            else:
                # two pieces per tensor -> two HWDGE queues in parallel;
                # chunks run descending, so the high halves go first
                ka = qk_pool.tile([P, SK // 2], F16, tag="ka", name="ka")
                kb = qk_pool.tile([P, SK // 2], F16, tag="kb", name="kb")
                qa = qk_pool.tile([P, SQ // 2], F16, tag="qa", name="qa")
                qb = qk_pool.tile([P, SQ // 2], F16, tag="qb", name="qb")
                nc.sync.dma_start(qb[:], qT_d[slot][:, SQ // 2 :])
                nc.sync.dma_start(ka[:], kT_d[slot][:, : SK // 2])
                nc.sync.dma_start(kb[:], kT_d[slot][:, SK // 2 :])
                nc.sync.dma_start(qa[:], qT_d[slot][:, : SQ // 2])
                vt = v_pool.tile([P, N_SK_TILES * (HN + 1)], F16, tag="v")
                nc.sync.dma_start(vt[:], v_d[slot])
                kslice = lambda j: (ka if j < 8 else kb)[
                    :, (j % 8) * P : (j % 8 + 1) * P
                ]
                qchunk = lambda ci: (qa if ci < 4 else qb)[
                    :, (ci % 4) * CHUNK : (ci % 4 + 1) * CHUNK
                ]
                vslice = lambda j: vt[:, j * (HN + 1) : (j + 1) * (HN + 1)]
            return kslice, qchunk, vslice

        # ---- emit one chunk's QK + exp into per-chunk groups ------------
        etmap = {}

        def emit_qk(slot, slot_io, ci):
            kslice, qchunk, _ = slot_io
            js = list(range(2 * ci + 2))     # ascending; diagonal j last
            diag = 2 * ci + 1
            for g0 in range(0, len(js), GROUP):
                gjs = js[g0 : g0 + GROUP]
                sc = sc_ps.tile([P, GROUP * CHUNK], F32, tag="scores", name="sc")
                gw = 0
                for k_idx, j in enumerate(gjs):
                    co = k_idx * CHUNK
                    w = P if j == diag else CHUNK
                    nc.tensor.matmul(
                        sc[:, co : co + w], kslice(j), qchunk(ci)[:, 0:w],
                        start=True, stop=True,
                    )
                    gw = co + w
                    # causal mask on the PE: sc[m, n] += triT[n%128, m].
                    # Must directly follow its QK matmul — start=False
                    # continues only the most recent accumulation group.
                    if j == diag:
                        nc.tensor.matmul(
                            sc[:, co : co + P], triT_sb[:], id2_sb[:, 0:P],
                            start=False, stop=True,
                        )
                    elif j == diag - 1:
                        nc.tensor.matmul(
                            sc[:, co + P : co + 2 * P], triT_sb[:], id2_sb[:, 0:P],
                            start=False, stop=True,
                        )
                et = e_pool.tile([P, GROUP * CHUNK], F16, tag="expT", name="et")
                nc.scalar.activation(
                    et[:, :gw], sc[:, :gw],
                    mybir.ActivationFunctionType.Exp,
                    scale=inv_norm,
                )
                for k_idx, j in enumerate(gjs):
                    etmap[(slot, ci, j)] = (et, k_idx * CHUNK)

        # ---- emit one chunk's PV + normalize + (maybe) out DMA ----------
        def emit_pv(slot, slot_io, ci, oq_tiles, done_quarters):
            _, _, vslice = slot_io
            exp_tiles = {j: etmap[(slot, ci, j)] for j in range(2 * ci + 2)}
            # one PSUM tile holds both context vectors of the chunk:
            # i_lo at cols [0,129), i_hi at cols [129,258)
            cx = cx_ps.tile([P, 2 * (HN + 1)], F32, tag="ctx")
            for i in (2 * ci + 1, 2 * ci):   # i_hi (first half of chunk), i_lo
                off = 0 if i == 2 * ci + 1 else P
                base = (HN + 1) if i == 2 * ci + 1 else 0
                pv_js = list(range(i + 1))
                for idx, j in enumerate(pv_js):
                    et, co = exp_tiles[j]
                    nc.tensor.matmul(
                        cx[:, base : base + HN + 1],
                        et[:, co + off : co + off + P], vslice(j),
                        start=(idx == 0), stop=(idx == len(pv_js) - 1),
                    )
            recip = r_pool.tile([P, 2], F32, tag="recip")
            nc.vector.reciprocal(
                recip[:], cx[:, HN : 2 * HN + 2 : HN + 1]
            )
            qt_idx = (2 * ci) // 4
            if qt_idx not in oq_tiles:
                oq_tiles[qt_idx] = o_pool.tile(
                    [P, 4 * HN], F32, tag="oq", name="oq"
                )
            ot = oq_tiles[qt_idx]
            col = (2 * ci % 4) * HN          # i_lo column; i_hi is the next one
            nc.vector.tensor_mul(
                ot[:, col : col + 2 * HN].rearrange("p (s c) -> p s c", s=2),
                cx[:].rearrange("p (s c) -> p s c", s=2)[:, :, 0:HN],
                recip[:].rearrange("p (s c) -> p s c", c=1).broadcast_to(
                    [P, 2, HN]
                ),
            )
            done_quarters.setdefault(qt_idx, set()).add(ci)
            if len(done_quarters[qt_idx]) == 2:
                nc.sync.dma_start(out_d[slot, qt_idx], oq_tiles[qt_idx][:])

        # ---- main schedule: PV runs as soon as its exp tiles exist ------
        pvq = []  # [(slot, slot_io, ci, oq_tiles, done_quarters)]

        def drain_pv(final=False):
            # keep one chunk pending (unless final) so PV trails the QK
            # stream and the PE never queues behind a just-issued exp
            while pvq and (final or len(pvq) >= 2):
                slot, slot_io, ci, oq, dq = pvq.pop(0)
                emit_pv(slot, slot_io, ci, oq, dq)

        # interleave slot boundaries: the next slot's big chunk 7 runs
        # between the previous slot's tiny chunks 1 and 0, so the exp
        # stream never starves while a new slot spins up
        sched = [("load", 0)] + [("chunk", 0, ci) for ci in (7, 6, 5, 4, 3, 2)]
        for s in range(SLOTS_PER_CORE - 1):
            sched += [("load", s + 1), ("chunk", s, 1), ("chunk", s + 1, 7),
                      ("chunk", s, 0)]
            tail = (6, 5, 4, 3, 2) if s + 1 < SLOTS_PER_CORE - 1 else \
                (6, 5, 4, 3, 2, 1, 0)
            sched += [("chunk", s + 1, ci) for ci in tail]

        slot_io_of = {}
        slot_state = {}
        for step in sched:
            if step[0] == "load":
                slot = step[1]
                slot_io_of[slot] = load_slot(slot)
                slot_state[slot] = ({}, {})  # oq_tiles, done_quarters
            else:
                _, slot, ci = step
                emit_qk(slot, slot_io_of[slot], ci)
                oq, dq = slot_state[slot]
                pvq.append((slot, slot_io_of[slot], ci, oq, dq))
                drain_pv()
        drain_pv(final=True)
        assert not pvq

    nc.compile()
    return nc


_cache = {}


def _get_program(mask: np.ndarray):
    # this kernel is specialized to the standard causal mask
    m = np.asarray(mask)
    causal = np.triu(np.ones((SQ, SK), dtype=bool), k=1)
    for b in range(B):
        if not np.array_equal(m[b, 0], causal):
            raise ValueError("kernel specialized to causal attention mask")
    if "nc" not in _cache:
        _cache["nc"] = _build_program()
    return _cache["nc"]


def _core_slots(c):
    return [(0, 2 * c), (0, 2 * c + 1), (1, 2 * c), (1, 2 * c + 1)]


def prepare(query_layer, key_layer, value_layer, attention_mask):
    q = np.asarray(query_layer)
    k = np.asarray(key_layer)
    v = np.asarray(value_layer)
    nc = _get_program(np.asarray(attention_mask))

    # qT with the two 128-col tiles of each 256 chunk swapped:
    # sbuf layout col (256*ci + [0..255]) = sq (256*ci + [128..255, 0..127])
    q16 = q.astype(np.float16)                      # [SQ, B, NP, HN]
    qv = q16.reshape(N_CHUNKS, 2, P, B, NP, HN)[:, ::-1]   # swap tile pairs
    qT_all = np.ascontiguousarray(qv.transpose(3, 4, 5, 0, 1, 2)).reshape(
        B, NP, HN, SQ
    )
    k16 = k.astype(np.float16)
    kT_all = np.ascontiguousarray(k16.transpose(1, 2, 3, 0))  # [B, NP, HN, SK]

    v5 = v.reshape(N_SK_TILES, P, B, NP, HN).transpose(2, 3, 1, 0, 4)
    v_aug_all = np.empty((B, NP, P, N_SK_TILES, HN + 1), dtype=np.float16)
    v_aug_all[..., :HN] = v5
    v_aug_all[..., HN] = 1.0
    v_aug_all = v_aug_all.reshape(B, NP, P, N_SK_TILES * (HN + 1))

    # mask-matmul constants: sc[m, n] += sum_p triT[p, m] * ident2[p, n]
    #   = triT[n%128, m]  which must be NEG where (n%128) < m
    triT = np.where(
        np.arange(P)[:, None] < np.arange(P)[None, :], NEG, 0.0
    ).astype(np.float16)                            # triT[p, c] = NEG if p < c
    ident2 = np.concatenate([np.eye(P), np.eye(P)], axis=1).astype(np.float16)

    in_maps = []
    for c in range(N_CORES):
        slots = _core_slots(c)
        im = {
            "qT": np.ascontiguousarray(np.stack([qT_all[b, n] for b, n in slots])),
            "kT": np.ascontiguousarray(np.stack([kT_all[b, n] for b, n in slots])),
            "v_aug": np.ascontiguousarray(
                np.stack([v_aug_all[b, n] for b, n in slots])
            ),
            "triT": triT,
            "ident2": ident2,
        }
        in_maps.append(im)
    return nc, in_maps


def assemble(results):
    """Gather per-core 'out' arrays into the full [SQ, B, NP*HN] output."""
    full = np.empty((SQ, B, NP * HN), dtype=np.float32)
    for c in range(N_CORES):
        o = results[c]["out"]  # [4, 4, 128, 512]
        for s, (b, n) in enumerate(_core_slots(c)):
            ctx = (
                o[s].reshape(4, P, 4, HN).transpose(0, 2, 1, 3).reshape(SQ, HN)
            )
            full[:, b, n * HN : (n + 1) * HN] = ctx
    return full


def kernel(query_layer, key_layer, value_layer, attention_mask):
    from concourse.bass_utils import run_bass_kernel_spmd

    nc, in_maps = prepare(query_layer, key_layer, value_layer, attention_mask)
    res = run_bass_kernel_spmd(nc, in_maps, list(range(N_CORES)))
    return assemble(res.results)


# revision 33
# speedup vs baseline: 1.0387x; 1.0387x over previous
"""Causal multi-head attention forward on 8 Trainium2 NeuronCores.

Problem: nn_CoreAttention (SQ=SK=2048, B=2, NP=16 heads, HN=128, fp32).

Sharding: 32 (batch, head) pairs split 4 per core (tensor-parallel over
heads, data-parallel over batch). No collectives.

Per (b, n) pair, in transposed score orientation (sk on partitions):
    scoresT[sk, sq] = (K Q^T)                 (PE matmul, fp16 in, hn contracted)
    expT = exp(scoresT * 1/sqrt(HN) + mask)   (ScalarE, fp16 out)
    ctx_aug[sq, hn+1] = expT^T @ [V | 1]      (PE matmul, sk contracted;
                                               col hn holds the softmax denom)
    ctx = ctx_aug[:, :hn] * 1/ctx_aug[:, hn]  (DVE reciprocal + scale)

v2 structure (vs the v1 baseline):
  - q/k cast to fp16 on host; no on-device casts; FWL stays enabled.
  - sq chunks of 256 with the two 128-tiles SWAPPED (host pre-swap), so the
    diagonal j-tile's masked half is the trailing half of the chunk: the
    diagonal QK matmul streams only 128 valid cols and exp skips the rest.
  - the two triangular tiles per chunk land on 256 contiguous score cols:
    one DVE add with a single [128,256] additive tile handles all masking.
  - PV is software-pipelined one chunk behind QK so the PE never stalls on
    the exp latency of the chunk it just produced.
  - slot 0 loads q/k/v in small pieces (fast start); the last slot walks
    chunks descending so the kernel tail is the smallest chunk.
  - outputs accumulate in SBUF and leave as 4 wide DMAs per slot.
"""

import math
import numpy as np
from contextlib import ExitStack

import concourse.bacc as bacc
import concourse.tile as tile
from concourse import mybir

SQ, SK, B, NP, HN = 2048, 2048, 2, 16, 128
N_CORES = 8
SLOTS_PER_CORE = 4
P = 128
CHUNK = 256
N_CHUNKS = SQ // CHUNK      # 8
N_SK_TILES = SK // P        # 16
NEG = -60000.0

import os
GROUP = int(os.environ.get("ATT_GROUP", "6"))          # j-tiles per PSUM group
SC_BUFS = int(os.environ.get("ATT_SC_BUFS", "2"))
CX_BUFS = int(os.environ.get("ATT_CX_BUFS", "2"))
E_BUFS = int(os.environ.get("ATT_E_BUFS", "8"))

F32 = mybir.dt.float32
F16 = mybir.dt.float16


def _build_program():
    nc = bacc.Bacc()

    qT_d = nc.declare_dram_parameter("qT", [SLOTS_PER_CORE, P, SQ], F16, isOutput=False)
    kT_d = nc.declare_dram_parameter("kT", [SLOTS_PER_CORE, P, SK], F16, isOutput=False)
    v_d = nc.declare_dram_parameter(
        "v_aug", [SLOTS_PER_CORE, P, N_SK_TILES * (HN + 1)], F16, isOutput=False
    )
    # triT[p, c] = NEG if p < c else 0; ident2 = [I | I]
    triT_d = nc.declare_dram_parameter("triT", [P, P], F16, isOutput=False)
    id2_d = nc.declare_dram_parameter("ident2", [P, 2 * P], F16, isOutput=False)
    out_d = nc.declare_dram_parameter(
        "out", [SLOTS_PER_CORE, 4, P, 4 * HN], F32, isOutput=True
    )

    inv_norm = 1.0 / math.sqrt(HN)

    with tile.TileContext(nc) as tc, ExitStack() as ctx:
        qk_pool = ctx.enter_context(tc.tile_pool(name="qk", bufs=2))
        v_pool = ctx.enter_context(tc.tile_pool(name="v", bufs=2))
        m_pool = ctx.enter_context(tc.tile_pool(name="m", bufs=1))
        e_pool = ctx.enter_context(tc.tile_pool(name="e", bufs=E_BUFS))
        o_pool = ctx.enter_context(tc.tile_pool(name="o", bufs=4))
        r_pool = ctx.enter_context(tc.tile_pool(name="r", bufs=4))
        sc_ps = ctx.enter_context(tc.tile_pool(name="sc", bufs=SC_BUFS, space="PSUM"))
        cx_ps = ctx.enter_context(tc.tile_pool(name="cx", bufs=CX_BUFS, space="PSUM"))

        triT_sb = m_pool.tile([P, P], F16, tag="triT")
        nc.scalar.dma_start(triT_sb[:], triT_d[:])
        id2_sb = m_pool.tile([P, 2 * P], F16, tag="id2")

        # touch Exp immediately so the ACT table loads during the initial DMAs
        warm_in = m_pool.tile([P, 1], F32, tag="warm_in")
        nc.vector.memset(warm_in[:], 0.0)
        warm_out = m_pool.tile([P, 1], F32, tag="warm_out")
        nc.scalar.activation(
            warm_out[:], warm_in[:], mybir.ActivationFunctionType.Exp
        )



        # ---- per-slot input loading -------------------------------------
        def load_slot(slot):
            """Returns (kslice, qchunk, vslice) accessor fns for this slot."""
            if slot == 0:
                # pieces, issued in first-use order (chunks run descending)
                # across both HWDGE queues (sync + scalar run in parallel and
                # each queue moves ~55 GB/s, so split k into 4 transfers)
                kts = [
                    qk_pool.tile([P, 4 * P], F16, tag=f"k{pc}", name=f"k{pc}")
                    for pc in range(4)
                ]
                qts = [
                    qk_pool.tile([P, CHUNK], F16, tag=f"q{pc}", name=f"q{pc}")
                    for pc in range(N_CHUNKS)
                ]
                vts = [
                    v_pool.tile([P, 8 * (HN + 1)], F16, tag=f"v{pc}", name=f"v{pc}")
                    for pc in range(2)
                ]
                nc.sync.dma_start(kts[0][:], kT_d[slot][:, 0:512])
                nc.scalar.dma_start(qts[7][:], qT_d[slot][:, 7 * CHUNK : 8 * CHUNK])
                nc.sync.dma_start(kts[1][:], kT_d[slot][:, 512:1024])
                nc.scalar.dma_start(kts[2][:], kT_d[slot][:, 1024:1536])
                nc.sync.dma_start(id2_sb[:], id2_d[:])
                nc.scalar.dma_start(kts[3][:], kT_d[slot][:, 1536:2048])
                nc.sync.dma_start(qts[6][:], qT_d[slot][:, 6 * CHUNK : 7 * CHUNK])
                for pc in range(2):
                    nc.scalar.dma_start(
                        vts[pc][:],
                        v_d[slot][:, pc * 8 * (HN + 1) : (pc + 1) * 8 * (HN + 1)],
                    )
                for pc in range(N_CHUNKS - 3, -1, -1):
                    nc.sync.dma_start(
                        qts[pc][:], qT_d[slot][:, pc * CHUNK : (pc + 1) * CHUNK]
                    )
                kslice = lambda j: kts[j // 4][:, (j % 4) * P : (j % 4 + 1) * P]
                qchunk = lambda ci: qts[ci][:]
                vslice = lambda j: vts[j // 8][
                    :, (j % 8) * (HN + 1) : (j % 8 + 1) * (HN + 1)
                ]
            else:
                # two pieces per tensor -> two HWDGE queues in parallel;
                # chunks run descending, so the high halves go first
                ka = qk_pool.tile([P, SK // 2], F16, tag="ka", name="ka")
                kb = qk_pool.tile([P, SK // 2], F16, tag="kb", name="kb")
                qa = qk_pool.tile([P, SQ // 2], F16, tag="qa", name="qa")
                qb = qk_pool.tile([P, SQ // 2], F16, tag="qb", name="qb")
                nc.sync.dma_start(qb[:], qT_d[slot][:, SQ // 2 :])
                nc.sync.dma_start(ka[:], kT_d[slot][:, : SK // 2])
                nc.sync.dma_start(kb[:], kT_d[slot][:, SK // 2 :])
                nc.sync.dma_start(qa[:], qT_d[slot][:, : SQ // 2])
                vt = v_pool.tile([P, N_SK_TILES * (HN + 1)], F16, tag="v")
                nc.sync.dma_start(vt[:], v_d[slot])
                kslice = lambda j: (ka if j < 8 else kb)[
                    :, (j % 8) * P : (j % 8 + 1) * P
                ]
                qchunk = lambda ci: (qa if ci < 4 else qb)[
                    :, (ci % 4) * CHUNK : (ci % 4 + 1) * CHUNK
                ]
                vslice = lambda j: vt[:, j * (HN + 1) : (j + 1) * (HN + 1)]
            return kslice, qchunk, vslice

        # ---- emit one chunk's QK + exp into per-chunk groups ------------
        etmap = {}

        def emit_qk(slot, slot_io, ci):
            kslice, qchunk, _ = slot_io
            js = list(range(2 * ci + 2))     # ascending; diagonal j last
            diag = 2 * ci + 1
            for g0 in range(0, len(js), GROUP):
                gjs = js[g0 : g0 + GROUP]
                sc = sc_ps.tile([P, GROUP * CHUNK], F32, tag="scores", name="sc")
                gw = 0
                for k_idx, j in enumerate(gjs):
                    co = k_idx * CHUNK
                    w = P if j == diag else CHUNK
                    nc.tensor.matmul(
                        sc[:, co : co + w], kslice(j), qchunk(ci)[:, 0:w],
                        start=True, stop=True,
                    )
                    gw = co + w
                    # causal mask on the PE: sc[m, n] += triT[n%128, m].
                    # Must directly follow its QK matmul — start=False
                    # continues only the most recent accumulation group.
                    if j == diag:
                        nc.tensor.matmul(
                            sc[:, co : co + P], triT_sb[:], id2_sb[:, 0:P],
                            start=False, stop=True,
                        )
                    elif j == diag - 1:
                        nc.tensor.matmul(
                            sc[:, co + P : co + 2 * P], triT_sb[:], id2_sb[:, 0:P],
                            start=False, stop=True,
                        )
                et = e_pool.tile([P, GROUP * CHUNK], F16, tag="expT", name="et")
                nc.scalar.activation(
                    et[:, :gw], sc[:, :gw],
                    mybir.ActivationFunctionType.Exp,
                    scale=inv_norm,
                )
                for k_idx, j in enumerate(gjs):
                    etmap[(slot, ci, j)] = (et, k_idx * CHUNK)

        # ---- emit one chunk's PV + normalize + (maybe) out DMA ----------
        def emit_pv(slot, slot_io, ci, oq_tiles, done_quarters):
            _, _, vslice = slot_io
            exp_tiles = {j: etmap[(slot, ci, j)] for j in range(2 * ci + 2)}
            # one PSUM tile holds both context vectors of the chunk:
            # i_lo at cols [0,129), i_hi at cols [129,258)
            cx = cx_ps.tile([P, 2 * (HN + 1)], F32, tag="ctx")
            for i in (2 * ci + 1, 2 * ci):   # i_hi (first half of chunk), i_lo
                off = 0 if i == 2 * ci + 1 else P
                base = (HN + 1) if i == 2 * ci + 1 else 0
                pv_js = list(range(i + 1))
                for idx, j in enumerate(pv_js):
                    et, co = exp_tiles[j]
                    nc.tensor.matmul(
                        cx[:, base : base + HN + 1],
                        et[:, co + off : co + off + P], vslice(j),
                        start=(idx == 0), stop=(idx == len(pv_js) - 1),
                    )
            recip = r_pool.tile([P, 2], F32, tag="recip")
            nc.vector.reciprocal(
                recip[:], cx[:, HN : 2 * HN + 2 : HN + 1]
            )
            qt_idx = (2 * ci) // 4
            if qt_idx not in oq_tiles:
                oq_tiles[qt_idx] = o_pool.tile(
                    [P, 4 * HN], F32, tag="oq", name="oq"
                )
            ot = oq_tiles[qt_idx]
            col = (2 * ci % 4) * HN          # i_lo column; i_hi is the next one
            nc.vector.tensor_mul(
                ot[:, col : col + 2 * HN].rearrange("p (s c) -> p s c", s=2),
                cx[:].rearrange("p (s c) -> p s c", s=2)[:, :, 0:HN],
                recip[:].rearrange("p (s c) -> p s c", c=1).broadcast_to(
                    [P, 2, HN]
                ),
            )
            done_quarters.setdefault(qt_idx, set()).add(ci)
            if len(done_quarters[qt_idx]) == 2:
                nc.sync.dma_start(out_d[slot, qt_idx], oq_tiles[qt_idx][:])

        # ---- main schedule: PV runs as soon as its exp tiles exist ------
        pvq = []  # [(slot, slot_io, ci, oq_tiles, done_quarters)]

        def drain_pv(final=False):
            # keep one chunk pending (unless final) so PV trails the QK
            # stream and the PE never queues behind a just-issued exp
            while pvq and (final or len(pvq) >= 2):
                slot, slot_io, ci, oq, dq = pvq.pop(0)
                emit_pv(slot, slot_io, ci, oq, dq)

        slot_state = {}
        for slot in range(SLOTS_PER_CORE):
            slot_io = load_slot(slot)
            slot_state[slot] = ({}, {})  # oq_tiles, done_quarters
            for ci in range(N_CHUNKS - 1, -1, -1):
                emit_qk(slot, slot_io, ci)
                oq, dq = slot_state[slot]
                pvq.append((slot, slot_io, ci, oq, dq))
                drain_pv()
        drain_pv(final=True)
        assert not pvq

    nc.compile()
    return nc


_cache = {}


def _get_program(mask: np.ndarray):
    # this kernel is specialized to the standard causal mask
    m = np.asarray(mask)
    causal = np.triu(np.ones((SQ, SK), dtype=bool), k=1)
    for b in range(B):
        if not np.array_equal(m[b, 0], causal):
            raise ValueError("kernel specialized to causal attention mask")
    if "nc" not in _cache:
        _cache["nc"] = _build_program()
    return _cache["nc"]


def _core_slots(c):
    return [(0, 2 * c), (0, 2 * c + 1), (1, 2 * c), (1, 2 * c + 1)]


def prepare(query_layer, key_layer, value_layer, attention_mask):
    q = np.asarray(query_layer)
    k = np.asarray(key_layer)
    v = np.asarray(value_layer)
    nc = _get_program(np.asarray(attention_mask))

    # qT with the two 128-col tiles of each 256 chunk swapped:
    # sbuf layout col (256*ci + [0..255]) = sq (256*ci + [128..255, 0..127])
    q16 = q.astype(np.float16)                      # [SQ, B, NP, HN]
    qv = q16.reshape(N_CHUNKS, 2, P, B, NP, HN)[:, ::-1]   # swap tile pairs
    qT_all = np.ascontiguousarray(qv.transpose(3, 4, 5, 0, 1, 2)).reshape(
        B, NP, HN, SQ
    )
    k16 = k.astype(np.float16)
    kT_all = np.ascontiguousarray(k16.transpose(1, 2, 3, 0))  # [B, NP, HN, SK]

    v5 = v.reshape(N_SK_TILES, P, B, NP, HN).transpose(2, 3, 1, 0, 4)
    v_aug_all = np.empty((B, NP, P, N_SK_TILES, HN + 1), dtype=np.float16)
    v_aug_all[..., :HN] = v5
    v_aug_all[..., HN] = 1.0
    v_aug_all = v_aug_all.reshape(B, NP, P, N_SK_TILES * (HN + 1))

    # mask-matmul constants: sc[m, n] += sum_p triT[p, m] * ident2[p, n]
    #   = triT[n%128, m]  which must be NEG where (n%128) < m
    triT = np.where(
        np.arange(P)[:, None] < np.arange(P)[None, :], NEG, 0.0
    ).astype(np.float16)                            # triT[p, c] = NEG if p < c
    ident2 = np.concatenate([np.eye(P), np.eye(P)], axis=1).astype(np.float16)

    in_maps = []
    for c in range(N_CORES):
        slots = _core_slots(c)
        im = {
            "qT": np.ascontiguousarray(np.stack([qT_all[b, n] for b, n in slots])),
            "kT": np.ascontiguousarray(np.stack([kT_all[b, n] for b, n in slots])),
            "v_aug": np.ascontiguousarray(
                np.stack([v_aug_all[b, n] for b, n in slots])
            ),
            "triT": triT,
            "ident2": ident2,
        }
        in_maps.append(im)
    return nc, in_maps


def assemble(results):
    """Gather per-core 'out' arrays into the full [SQ, B, NP*HN] output."""
    full = np.empty((SQ, B, NP * HN), dtype=np.float32)
    for c in range(N_CORES):
        o = results[c]["out"]  # [4, 4, 128, 512]
        for s, (b, n) in enumerate(_core_slots(c)):
            ctx = (
                o[s].reshape(4, P, 4, HN).transpose(0, 2, 1, 3).reshape(SQ, HN)
            )
            full[:, b, n * HN : (n + 1) * HN] = ctx
    return full


def kernel(query_layer, key_layer, value_layer, attention_mask):
    from concourse.bass_utils import run_bass_kernel_spmd

    nc, in_maps = prepare(query_layer, key_layer, value_layer, attention_mask)
    res = run_bass_kernel_spmd(nc, in_maps, list(range(N_CORES)))
    return assemble(res.results)


# revision 34
# speedup vs baseline: 1.2196x; 1.1741x over previous
"""Causal multi-head attention forward on 8 Trainium2 NeuronCores.

Problem: nn_CoreAttention (SQ=SK=2048, B=2, NP=16 heads, HN=128, fp32).

Sharding: 32 (batch, head) pairs split 4 per core (tensor-parallel over
heads, data-parallel over batch). No collectives.

Per (b, n) pair, in transposed score orientation (sk on partitions):
    scoresT[sk, sq] = (K Q^T)                 (PE matmul, fp16 in, hn contracted)
    expT = exp(scoresT * 1/sqrt(HN) + mask)   (ScalarE, fp16 out)
    ctx_aug[sq, hn+1] = expT^T @ [V | 1]      (PE matmul, sk contracted;
                                               col hn holds the softmax denom)
    ctx = ctx_aug[:, :hn] * 1/ctx_aug[:, hn]  (DVE reciprocal + scale)

v2 structure (vs the v1 baseline):
  - q/k cast to fp16 on host; no on-device casts; FWL stays enabled.
  - sq chunks of 256 with the two 128-tiles SWAPPED (host pre-swap), so the
    diagonal j-tile's masked half is the trailing half of the chunk: the
    diagonal QK matmul streams only 128 valid cols and exp skips the rest.
  - the two triangular tiles per chunk land on 256 contiguous score cols:
    one DVE add with a single [128,256] additive tile handles all masking.
  - PV is software-pipelined one chunk behind QK so the PE never stalls on
    the exp latency of the chunk it just produced.
  - slot 0 loads q/k/v in small pieces (fast start); the last slot walks
    chunks descending so the kernel tail is the smallest chunk.
  - outputs accumulate in SBUF and leave as 4 wide DMAs per slot.
"""

import math
import numpy as np
from contextlib import ExitStack

import concourse.bacc as bacc
import concourse.tile as tile
from concourse import mybir

SQ, SK, B, NP, HN = 2048, 2048, 2, 16, 128
N_CORES = 8
SLOTS_PER_CORE = 4
P = 128
CHUNK = 256
N_CHUNKS = SQ // CHUNK      # 8
N_SK_TILES = SK // P        # 16
NEG = -60000.0

import os
GROUP = int(os.environ.get("ATT_GROUP", "6"))          # j-tiles per PSUM group
SC_BUFS = int(os.environ.get("ATT_SC_BUFS", "2"))
CX_BUFS = int(os.environ.get("ATT_CX_BUFS", "2"))
E_BUFS = int(os.environ.get("ATT_E_BUFS", "8"))

F32 = mybir.dt.float32
F16 = mybir.dt.float16


def _build_program():
    nc = bacc.Bacc()

    qT_d = nc.declare_dram_parameter("qT", [SLOTS_PER_CORE, P, SQ], F16, isOutput=False)
    kT_d = nc.declare_dram_parameter("kT", [SLOTS_PER_CORE, P, SK], F16, isOutput=False)
    v_d = nc.declare_dram_parameter(
        "v_aug", [SLOTS_PER_CORE, P, N_SK_TILES * (HN + 1)], F16, isOutput=False
    )
    # triT[p, c] = NEG if p < c else 0; ident2 = [I | I]
    triT_d = nc.declare_dram_parameter("triT", [P, P], F16, isOutput=False)
    id2_d = nc.declare_dram_parameter("ident2", [P, 2 * P], F16, isOutput=False)
    out_d = nc.declare_dram_parameter(
        "out", [SLOTS_PER_CORE, 4, P, 4 * HN], F32, isOutput=True
    )

    inv_norm = 1.0 / math.sqrt(HN)

    with tile.TileContext(nc) as tc, ExitStack() as ctx:
        qk_pool = ctx.enter_context(tc.tile_pool(name="qk", bufs=2))
        v_pool = ctx.enter_context(tc.tile_pool(name="v", bufs=2))
        m_pool = ctx.enter_context(tc.tile_pool(name="m", bufs=1))
        e_pool = ctx.enter_context(tc.tile_pool(name="e", bufs=E_BUFS))
        o_pool = ctx.enter_context(tc.tile_pool(name="o", bufs=4))
        r_pool = ctx.enter_context(tc.tile_pool(name="r", bufs=4))
        sc_ps = ctx.enter_context(tc.tile_pool(name="sc", bufs=SC_BUFS, space="PSUM"))
        cx_ps = ctx.enter_context(tc.tile_pool(name="cx", bufs=CX_BUFS, space="PSUM"))

        triT_sb = m_pool.tile([P, P], F16, tag="triT")
        nc.scalar.dma_start(triT_sb[:], triT_d[:])
        id2_sb = m_pool.tile([P, 2 * P], F16, tag="id2")

        # touch Exp immediately so the ACT table loads during the initial DMAs
        warm_in = m_pool.tile([P, 1], F32, tag="warm_in")
        nc.vector.memset(warm_in[:], 0.0)
        warm_out = m_pool.tile([P, 1], F32, tag="warm_out")
        nc.scalar.activation(
            warm_out[:], warm_in[:], mybir.ActivationFunctionType.Exp
        )



        # ---- per-slot input loading -------------------------------------
        def load_slot(slot):
            """Returns (kslice, qchunk, vslice) accessor fns for this slot."""
            if slot == 0:
                # pieces, issued in first-use order (chunks run descending)
                # across both HWDGE queues (sync + scalar run in parallel and
                # each queue moves ~55 GB/s, so split k into 4 transfers)
                kts = [
                    qk_pool.tile([P, 4 * P], F16, tag=f"k{pc}", name=f"k{pc}")
                    for pc in range(4)
                ]
                qts = [
                    qk_pool.tile([P, CHUNK], F16, tag=f"q{pc}", name=f"q{pc}")
                    for pc in range(N_CHUNKS)
                ]
                vts = [
                    v_pool.tile([P, 8 * (HN + 1)], F16, tag=f"v{pc}", name=f"v{pc}")
                    for pc in range(2)
                ]
                nc.sync.dma_start(kts[0][:], kT_d[slot][:, 0:512])
                nc.scalar.dma_start(qts[7][:], qT_d[slot][:, 7 * CHUNK : 8 * CHUNK])
                nc.sync.dma_start(kts[1][:], kT_d[slot][:, 512:1024])
                nc.scalar.dma_start(kts[2][:], kT_d[slot][:, 1024:1536])
                nc.sync.dma_start(id2_sb[:], id2_d[:])
                nc.scalar.dma_start(kts[3][:], kT_d[slot][:, 1536:2048])
                nc.sync.dma_start(qts[6][:], qT_d[slot][:, 6 * CHUNK : 7 * CHUNK])
                for pc in range(2):
                    nc.scalar.dma_start(
                        vts[pc][:],
                        v_d[slot][:, pc * 8 * (HN + 1) : (pc + 1) * 8 * (HN + 1)],
                    )
                for pc in range(N_CHUNKS - 3, -1, -1):
                    nc.sync.dma_start(
                        qts[pc][:], qT_d[slot][:, pc * CHUNK : (pc + 1) * CHUNK]
                    )
                kslice = lambda j: kts[j // 4][:, (j % 4) * P : (j % 4 + 1) * P]
                qchunk = lambda ci: qts[ci][:]
                vslice = lambda j: vts[j // 8][
                    :, (j % 8) * (HN + 1) : (j % 8 + 1) * (HN + 1)
                ]
            else:
                kt = qk_pool.tile([P, SK], F16, tag="k")
                nc.sync.dma_start(kt[:], kT_d[slot])
                qt = qk_pool.tile([P, SQ], F16, tag="q")
                nc.sync.dma_start(qt[:], qT_d[slot])
                vt = v_pool.tile([P, N_SK_TILES * (HN + 1)], F16, tag="v")
                nc.sync.dma_start(vt[:], v_d[slot])
                kslice = lambda j: kt[:, j * P : (j + 1) * P]
                qchunk = lambda ci: qt[:, ci * CHUNK : (ci + 1) * CHUNK]
                vslice = lambda j: vt[:, j * (HN + 1) : (j + 1) * (HN + 1)]
            return kslice, qchunk, vslice

        # ---- emit one chunk's QK + exp into per-chunk groups ------------
        etmap = {}

        def emit_qk(slot, slot_io, ci):
            kslice, qchunk, _ = slot_io
            js = list(range(2 * ci + 2))     # ascending; diagonal j last
            diag = 2 * ci + 1
            for g0 in range(0, len(js), GROUP):
                gjs = js[g0 : g0 + GROUP]
                sc = sc_ps.tile([P, GROUP * CHUNK], F32, tag="scores", name="sc")
                gw = 0
                for k_idx, j in enumerate(gjs):
                    co = k_idx * CHUNK
                    w = P if j == diag else CHUNK
                    nc.tensor.matmul(
                        sc[:, co : co + w], kslice(j), qchunk(ci)[:, 0:w],
                        start=True, stop=True,
                    )
                    gw = co + w
                    # causal mask on the PE: sc[m, n] += triT[n%128, m].
                    # Must directly follow its QK matmul — start=False
                    # continues only the most recent accumulation group.
                    if j == diag:
                        nc.tensor.matmul(
                            sc[:, co : co + P], triT_sb[:], id2_sb[:, 0:P],
                            start=False, stop=True,
                        )
                    elif j == diag - 1:
                        nc.tensor.matmul(
                            sc[:, co + P : co + 2 * P], triT_sb[:], id2_sb[:, 0:P],
                            start=False, stop=True,
                        )
                et = e_pool.tile([P, GROUP * CHUNK], F16, tag="expT", name="et")
                nc.scalar.activation(
                    et[:, :gw], sc[:, :gw],
                    mybir.ActivationFunctionType.Exp,
                    scale=inv_norm,
                )
                for k_idx, j in enumerate(gjs):
                    etmap[(slot, ci, j)] = (et, k_idx * CHUNK)

        # ---- emit one chunk's PV + normalize + (maybe) out DMA ----------
        def emit_pv(slot, slot_io, ci, oq_tiles, done_quarters):
            _, _, vslice = slot_io
            exp_tiles = {j: etmap[(slot, ci, j)] for j in range(2 * ci + 2)}
            # one PSUM tile holds both context vectors of the chunk:
            # i_lo at cols [0,129), i_hi at cols [129,258)
            cx = cx_ps.tile([P, 2 * (HN + 1)], F32, tag="ctx")
            for i in (2 * ci + 1, 2 * ci):   # i_hi (first half of chunk), i_lo
                off = 0 if i == 2 * ci + 1 else P
                base = (HN + 1) if i == 2 * ci + 1 else 0
                pv_js = list(range(i + 1))
                for idx, j in enumerate(pv_js):
                    et, co = exp_tiles[j]
                    nc.tensor.matmul(
                        cx[:, base : base + HN + 1],
                        et[:, co + off : co + off + P], vslice(j),
                        start=(idx == 0), stop=(idx == len(pv_js) - 1),
                    )
            recip = r_pool.tile([P, 2], F32, tag="recip")
            nc.vector.reciprocal(
                recip[:], cx[:, HN : 2 * HN + 2 : HN + 1]
            )
            qt_idx = (2 * ci) // 4
            if qt_idx not in oq_tiles:
                oq_tiles[qt_idx] = o_pool.tile(
                    [P, 4 * HN], F32, tag="oq", name="oq"
                )
            ot = oq_tiles[qt_idx]
            col = (2 * ci % 4) * HN          # i_lo column; i_hi is the next one
            nc.vector.tensor_mul(
                ot[:, col : col + 2 * HN].rearrange("p (s c) -> p s c", s=2),
                cx[:].rearrange("p (s c) -> p s c", s=2)[:, :, 0:HN],
                recip[:].rearrange("p (s c) -> p s c", c=1).broadcast_to(
                    [P, 2, HN]
                ),
            )
            done_quarters.setdefault(qt_idx, set()).add(ci)
            if len(done_quarters[qt_idx]) == 2:
                nc.sync.dma_start(out_d[slot, qt_idx], oq_tiles[qt_idx][:])

        # ---- main schedule: PV runs as soon as its exp tiles exist ------
        pvq = []  # [(slot, slot_io, ci, oq_tiles, done_quarters)]

        def drain_pv(final=False):
            # keep one chunk pending (unless final) so PV trails the QK
            # stream and the PE never queues behind a just-issued exp
            while pvq and (final or len(pvq) >= 2):
                slot, slot_io, ci, oq, dq = pvq.pop(0)
                emit_pv(slot, slot_io, ci, oq, dq)

        slot_state = {}
        for slot in range(SLOTS_PER_CORE):
            slot_io = load_slot(slot)
            slot_state[slot] = ({}, {})  # oq_tiles, done_quarters
            for ci in range(N_CHUNKS - 1, -1, -1):
                emit_qk(slot, slot_io, ci)
                oq, dq = slot_state[slot]
                pvq.append((slot, slot_io, ci, oq, dq))
                drain_pv()
        drain_pv(final=True)
        assert not pvq

    nc.compile()
    return nc


_cache = {}


def _get_program(mask: np.ndarray):
    # this kernel is specialized to the standard causal mask
    m = np.asarray(mask)
    causal = np.triu(np.ones((SQ, SK), dtype=bool), k=1)
    for b in range(B):
        if not np.array_equal(m[b, 0], causal):
            raise ValueError("kernel specialized to causal attention mask")
    if "nc" not in _cache:
        _cache["nc"] = _build_program()
    return _cache["nc"]


def _core_slots(c):
    return [(0, 2 * c), (0, 2 * c + 1), (1, 2 * c), (1, 2 * c + 1)]


def prepare(query_layer, key_layer, value_layer, attention_mask):
    q = np.asarray(query_layer)
    k = np.asarray(key_layer)
    v = np.asarray(value_layer)
    nc = _get_program(np.asarray(attention_mask))

    # qT with the two 128-col tiles of each 256 chunk swapped:
    # sbuf layout col (256*ci + [0..255]) = sq (256*ci + [128..255, 0..127])
    q16 = q.astype(np.float16)                      # [SQ, B, NP, HN]
    qv = q16.reshape(N_CHUNKS, 2, P, B, NP, HN)[:, ::-1]   # swap tile pairs
    qT_all = np.ascontiguousarray(qv.transpose(3, 4, 5, 0, 1, 2)).reshape(
        B, NP, HN, SQ
    )
    k16 = k.astype(np.float16)
    kT_all = np.ascontiguousarray(k16.transpose(1, 2, 3, 0))  # [B, NP, HN, SK]

    v5 = v.reshape(N_SK_TILES, P, B, NP, HN).transpose(2, 3, 1, 0, 4)
    v_aug_all = np.empty((B, NP, P, N_SK_TILES, HN + 1), dtype=np.float16)
    v_aug_all[..., :HN] = v5
    v_aug_all[..., HN] = 1.0
    v_aug_all = v_aug_all.reshape(B, NP, P, N_SK_TILES * (HN + 1))

    # mask-matmul constants: sc[m, n] += sum_p triT[p, m] * ident2[p, n]
    #   = triT[n%128, m]  which must be NEG where (n%128) < m
    triT = np.where(
        np.arange(P)[:, None] < np.arange(P)[None, :], NEG, 0.0
    ).astype(np.float16)                            # triT[p, c] = NEG if p < c
    ident2 = np.concatenate([np.eye(P), np.eye(P)], axis=1).astype(np.float16)

    in_maps = []
    for c in range(N_CORES):
        slots = _core_slots(c)
        im = {
            "qT": np.ascontiguousarray(np.stack([qT_all[b, n] for b, n in slots])),
            "kT": np.ascontiguousarray(np.stack([kT_all[b, n] for b, n in slots])),
            "v_aug": np.ascontiguousarray(
                np.stack([v_aug_all[b, n] for b, n in slots])
            ),
            "triT": triT,
            "ident2": ident2,
        }
        in_maps.append(im)
    return nc, in_maps


def assemble(results):
    """Gather per-core 'out' arrays into the full [SQ, B, NP*HN] output."""
    full = np.empty((SQ, B, NP * HN), dtype=np.float32)
    for c in range(N_CORES):
        o = results[c]["out"]  # [4, 4, 128, 512]
        for s, (b, n) in enumerate(_core_slots(c)):
            ctx = (
                o[s].reshape(4, P, 4, HN).transpose(0, 2, 1, 3).reshape(SQ, HN)
            )
            full[:, b, n * HN : (n + 1) * HN] = ctx
    return full


def kernel(query_layer, key_layer, value_layer, attention_mask):
    from concourse.bass_utils import run_bass_kernel_spmd

    nc, in_maps = prepare(query_layer, key_layer, value_layer, attention_mask)
    res = run_bass_kernel_spmd(nc, in_maps, list(range(N_CORES)))
    return assemble(res.results)


# revision 38
# speedup vs baseline: 1.2358x; 1.0133x over previous
"""Causal multi-head attention forward on 8 Trainium2 NeuronCores.

Problem: nn_CoreAttention (SQ=SK=2048, B=2, NP=16 heads, HN=128, fp32).

Sharding: 32 (batch, head) pairs split 4 per core (tensor-parallel over
heads, data-parallel over batch). No collectives.

Per (b, n) pair, in transposed score orientation (sk on partitions):
    scoresT[sk, sq] = (K Q^T)                 (PE matmul, fp16 in, hn contracted)
    expT = exp(scoresT * 1/sqrt(HN) + mask)   (ScalarE, fp16 out)
    ctx_aug[sq, hn+1] = expT^T @ [V | 1]      (PE matmul, sk contracted;
                                               col hn holds the softmax denom)
    ctx = ctx_aug[:, :hn] * 1/ctx_aug[:, hn]  (DVE reciprocal + scale)

v2 structure (vs the v1 baseline):
  - q/k cast to fp16 on host; no on-device casts; FWL stays enabled.
  - sq chunks of 256 with the two 128-tiles SWAPPED (host pre-swap), so the
    diagonal j-tile's masked half is the trailing half of the chunk: the
    diagonal QK matmul streams only 128 valid cols and exp skips the rest.
  - the two triangular tiles per chunk land on 256 contiguous score cols:
    one DVE add with a single [128,256] additive tile handles all masking.
  - PV is software-pipelined one chunk behind QK so the PE never stalls on
    the exp latency of the chunk it just produced.
  - slot 0 loads q/k/v in small pieces (fast start); the last slot walks
    chunks descending so the kernel tail is the smallest chunk.
  - outputs accumulate in SBUF and leave as 4 wide DMAs per slot.
"""

import math
import numpy as np
from contextlib import ExitStack

import concourse.bacc as bacc
import concourse.tile as tile
from concourse import mybir

SQ, SK, B, NP, HN = 2048, 2048, 2, 16, 128
N_CORES = 8
SLOTS_PER_CORE = 4
P = 128
CHUNK = 256
N_CHUNKS = SQ // CHUNK      # 8
N_SK_TILES = SK // P        # 16
NEG = -60000.0

import os
GROUP = int(os.environ.get("ATT_GROUP", "6"))          # j-tiles per PSUM group
SC_BUFS = int(os.environ.get("ATT_SC_BUFS", "2"))
CX_BUFS = int(os.environ.get("ATT_CX_BUFS", "2"))
E_BUFS = int(os.environ.get("ATT_E_BUFS", "8"))

F32 = mybir.dt.float32
F16 = mybir.dt.float16


def _build_program():
    nc = bacc.Bacc()

    qT_d = nc.declare_dram_parameter("qT", [SLOTS_PER_CORE, P, SQ], F16, isOutput=False)
    kT_d = nc.declare_dram_parameter("kT", [SLOTS_PER_CORE, P, SK], F16, isOutput=False)
    v_d = nc.declare_dram_parameter(
        "v_aug", [SLOTS_PER_CORE, P, N_SK_TILES * (HN + 1)], F16, isOutput=False
    )
    # triT[p, c] = NEG if p < c else 0; ident2 = [I | I]
    triT_d = nc.declare_dram_parameter("triT", [P, P], F16, isOutput=False)
    id2_d = nc.declare_dram_parameter("ident2", [P, 2 * P], F16, isOutput=False)
    out_d = nc.declare_dram_parameter(
        "out", [SLOTS_PER_CORE, 4, P, 4 * HN], F32, isOutput=True
    )

    inv_norm = 1.0 / math.sqrt(HN)

    with tile.TileContext(nc) as tc, ExitStack() as ctx:
        qk_pool = ctx.enter_context(tc.tile_pool(name="qk", bufs=2))
        v_pool = ctx.enter_context(tc.tile_pool(name="v", bufs=2))
        m_pool = ctx.enter_context(tc.tile_pool(name="m", bufs=1))
        e_pool = ctx.enter_context(tc.tile_pool(name="e", bufs=E_BUFS))
        o_pool = ctx.enter_context(tc.tile_pool(name="o", bufs=4))
        r_pool = ctx.enter_context(tc.tile_pool(name="r", bufs=4))
        sc_ps = ctx.enter_context(tc.tile_pool(name="sc", bufs=SC_BUFS, space="PSUM"))
        cx_ps = ctx.enter_context(tc.tile_pool(name="cx", bufs=CX_BUFS, space="PSUM"))

        triT_sb = m_pool.tile([P, P], F16, tag="triT")
        nc.scalar.dma_start(triT_sb[:], triT_d[:])
        id2_sb = m_pool.tile([P, 2 * P], F16, tag="id2")

        # touch Exp immediately so the ACT table loads during the initial DMAs
        warm_in = m_pool.tile([P, 1], F32, tag="warm_in")
        nc.vector.memset(warm_in[:], 0.0)
        warm_out = m_pool.tile([P, 1], F32, tag="warm_out")
        nc.scalar.activation(
            warm_out[:], warm_in[:], mybir.ActivationFunctionType.Exp
        )



        # ---- per-slot input loading -------------------------------------
        def load_slot(slot):
            """Returns (kslice, qchunk, vslice) accessor fns for this slot."""
            if slot == 0:
                # pieces, issued in first-use order (chunks run descending)
                # across both HWDGE queues (sync + scalar run in parallel and
                # each queue moves ~55 GB/s, so split k into 4 transfers)
                kts = [
                    qk_pool.tile([P, 4 * P], F16, tag=f"k{pc}", name=f"k{pc}")
                    for pc in range(4)
                ]
                qts = [
                    qk_pool.tile([P, CHUNK], F16, tag=f"q{pc}", name=f"q{pc}")
                    for pc in range(N_CHUNKS)
                ]
                vts = [
                    v_pool.tile([P, 8 * (HN + 1)], F16, tag=f"v{pc}", name=f"v{pc}")
                    for pc in range(2)
                ]
                nc.sync.dma_start(kts[0][:], kT_d[slot][:, 0:512])
                nc.scalar.dma_start(qts[7][:], qT_d[slot][:, 7 * CHUNK : 8 * CHUNK])
                nc.sync.dma_start(kts[1][:], kT_d[slot][:, 512:1024])
                nc.scalar.dma_start(kts[2][:], kT_d[slot][:, 1024:1536])
                nc.sync.dma_start(id2_sb[:], id2_d[:])
                nc.scalar.dma_start(kts[3][:], kT_d[slot][:, 1536:2048])
                nc.sync.dma_start(qts[6][:], qT_d[slot][:, 6 * CHUNK : 7 * CHUNK])
                for pc in range(2):
                    nc.scalar.dma_start(
                        vts[pc][:],
                        v_d[slot][:, pc * 8 * (HN + 1) : (pc + 1) * 8 * (HN + 1)],
                    )
                for pc in range(N_CHUNKS - 3, -1, -1):
                    nc.sync.dma_start(
                        qts[pc][:], qT_d[slot][:, pc * CHUNK : (pc + 1) * CHUNK]
                    )
                kslice = lambda j: kts[j // 4][:, (j % 4) * P : (j % 4 + 1) * P]
                qchunk = lambda ci: qts[ci][:]
                vslice = lambda j: vts[j // 8][
                    :, (j % 8) * (HN + 1) : (j % 8 + 1) * (HN + 1)
                ]
            else:
                kt = qk_pool.tile([P, SK], F16, tag="k")
                nc.sync.dma_start(kt[:], kT_d[slot])
                qt = qk_pool.tile([P, SQ], F16, tag="q")
                nc.sync.dma_start(qt[:], qT_d[slot])
                vt = v_pool.tile([P, N_SK_TILES * (HN + 1)], F16, tag="v")
                nc.sync.dma_start(vt[:], v_d[slot])
                kslice = lambda j: kt[:, j * P : (j + 1) * P]
                qchunk = lambda ci: qt[:, ci * CHUNK : (ci + 1) * CHUNK]
                vslice = lambda j: vt[:, j * (HN + 1) : (j + 1) * (HN + 1)]
            return kslice, qchunk, vslice

        # ---- score packer (within a slot): QK blocks stream into shared
        # 1536-col PSUM tiles so most exp activations run at max width;
        # flushed at slot boundaries to avoid cross-slot coupling
        CAP = GROUP * CHUNK
        etmap = {}
        packer = {"sc": None, "fill": 0, "entries": []}

        def flush_packer():
            if packer["sc"] is None or packer["fill"] == 0:
                return
            et = e_pool.tile([P, CAP], F16, tag="expT", name="et")
            nc.scalar.activation(
                et[:, : packer["fill"]], packer["sc"][:, : packer["fill"]],
                mybir.ActivationFunctionType.Exp,
                scale=inv_norm,
            )
            for key, off in packer["entries"]:
                etmap[key] = (et, off)
            packer["sc"] = None
            packer["fill"] = 0
            packer["entries"] = []

        def emit_qk(slot, slot_io, ci):
            kslice, qchunk, _ = slot_io
            diag = 2 * ci + 1
            for j in range(2 * ci + 2):      # ascending; diagonal j last
                w = P if j == diag else CHUNK
                if packer["sc"] is None or packer["fill"] + w > CAP:
                    flush_packer()
                if packer["sc"] is None:
                    packer["sc"] = sc_ps.tile(
                        [P, CAP], F32, tag="scores", name="sc"
                    )
                sc, co = packer["sc"], packer["fill"]
                nc.tensor.matmul(
                    sc[:, co : co + w], kslice(j), qchunk(ci)[:, 0:w],
                    start=True, stop=True,
                )
                # causal mask on the PE: sc[m, n] += triT[n%128, m].
                # Must directly follow its QK matmul — start=False
                # continues only the most recent accumulation group.
                if j == diag:
                    nc.tensor.matmul(
                        sc[:, co : co + P], triT_sb[:], id2_sb[:, 0:P],
                        start=False, stop=True,
                    )
                elif j == diag - 1:
                    nc.tensor.matmul(
                        sc[:, co + P : co + 2 * P], triT_sb[:], id2_sb[:, 0:P],
                        start=False, stop=True,
                    )
                packer["entries"].append(((slot, ci, j), co))
                packer["fill"] = co + w

        # ---- emit one chunk's PV + normalize + (maybe) out DMA ----------
        def emit_pv(slot, slot_io, ci, oq_tiles, done_quarters):
            _, _, vslice = slot_io
            exp_tiles = {j: etmap[(slot, ci, j)] for j in range(2 * ci + 2)}
            # one PSUM tile holds both context vectors of the chunk:
            # i_lo at cols [0,129), i_hi at cols [129,258)
            cx = cx_ps.tile([P, 2 * (HN + 1)], F32, tag="ctx")
            for i in (2 * ci + 1, 2 * ci):   # i_hi (first half of chunk), i_lo
                off = 0 if i == 2 * ci + 1 else P
                base = (HN + 1) if i == 2 * ci + 1 else 0
                pv_js = list(range(i + 1))
                for idx, j in enumerate(pv_js):
                    et, co = exp_tiles[j]
                    nc.tensor.matmul(
                        cx[:, base : base + HN + 1],
                        et[:, co + off : co + off + P], vslice(j),
                        start=(idx == 0), stop=(idx == len(pv_js) - 1),
                    )
            recip = r_pool.tile([P, 2], F32, tag="recip")
            nc.vector.reciprocal(
                recip[:], cx[:, HN : 2 * HN + 2 : HN + 1]
            )
            qt_idx = (2 * ci) // 4
            if qt_idx not in oq_tiles:
                oq_tiles[qt_idx] = o_pool.tile(
                    [P, 4 * HN], F32, tag="oq", name="oq"
                )
            ot = oq_tiles[qt_idx]
            col = (2 * ci % 4) * HN          # i_lo column; i_hi is the next one
            nc.vector.tensor_mul(
                ot[:, col : col + 2 * HN].rearrange("p (s c) -> p s c", s=2),
                cx[:].rearrange("p (s c) -> p s c", s=2)[:, :, 0:HN],
                recip[:].rearrange("p (s c) -> p s c", c=1).broadcast_to(
                    [P, 2, HN]
                ),
            )
            if slot == SLOTS_PER_CORE - 1:
                # last slot: ship each chunk's half-quarter as soon as it is
                # normalized, so the final DMA (and the epilogue's DMA drain)
                # starts as early as possible
                h = ci % 2
                nc.sync.dma_start(
                    out_d[slot, qt_idx][:, h * 2 * HN : (h + 1) * 2 * HN],
                    ot[:, h * 2 * HN : (h + 1) * 2 * HN],
                )
            else:
                done_quarters.setdefault(qt_idx, set()).add(ci)
                if len(done_quarters[qt_idx]) == 2:
                    nc.sync.dma_start(out_d[slot, qt_idx], oq_tiles[qt_idx][:])

        # ---- main schedule: PV runs as soon as its exp tiles exist ------
        pvq = []  # [(slot, slot_io, ci, oq_tiles, done_quarters)]

        def drain_pv(final=False):
            # keep one chunk pending (unless final) so PV trails the QK
            # stream and the PE never queues behind a just-issued exp;
            # a chunk is ready once its diagonal block has been exp'd
            while pvq and (final or len(pvq) >= 2):
                slot, slot_io, ci, oq, dq = pvq[0]
                if (slot, ci, 2 * ci + 1) not in etmap:
                    return
                pvq.pop(0)
                emit_pv(slot, slot_io, ci, oq, dq)

        slot_state = {}
        for slot in range(SLOTS_PER_CORE):
            slot_io = load_slot(slot)
            slot_state[slot] = ({}, {})  # oq_tiles, done_quarters
            for ci in range(N_CHUNKS - 1, -1, -1):
                emit_qk(slot, slot_io, ci)
                oq, dq = slot_state[slot]
                pvq.append((slot, slot_io, ci, oq, dq))
                drain_pv()
            flush_packer()   # keep exp tiles slot-local
            drain_pv()
        drain_pv(final=True)
        assert not pvq

    nc.compile()
    return nc


_cache = {}


def _get_program(mask: np.ndarray):
    # this kernel is specialized to the standard causal mask
    m = np.asarray(mask)
    causal = np.triu(np.ones((SQ, SK), dtype=bool), k=1)
    for b in range(B):
        if not np.array_equal(m[b, 0], causal):
            raise ValueError("kernel specialized to causal attention mask")
    if "nc" not in _cache:
        _cache["nc"] = _build_program()
    return _cache["nc"]


def _core_slots(c):
    return [(0, 2 * c), (0, 2 * c + 1), (1, 2 * c), (1, 2 * c + 1)]


def prepare(query_layer, key_layer, value_layer, attention_mask):
    q = np.asarray(query_layer)
    k = np.asarray(key_layer)
    v = np.asarray(value_layer)
    nc = _get_program(np.asarray(attention_mask))

    # qT with the two 128-col tiles of each 256 chunk swapped:
    # sbuf layout col (256*ci + [0..255]) = sq (256*ci + [128..255, 0..127])
    q16 = q.astype(np.float16)                      # [SQ, B, NP, HN]
    qv = q16.reshape(N_CHUNKS, 2, P, B, NP, HN)[:, ::-1]   # swap tile pairs
    qT_all = np.ascontiguousarray(qv.transpose(3, 4, 5, 0, 1, 2)).reshape(
        B, NP, HN, SQ
    )
    k16 = k.astype(np.float16)
    kT_all = np.ascontiguousarray(k16.transpose(1, 2, 3, 0))  # [B, NP, HN, SK]

    v5 = v.reshape(N_SK_TILES, P, B, NP, HN).transpose(2, 3, 1, 0, 4)
    v_aug_all = np.empty((B, NP, P, N_SK_TILES, HN + 1), dtype=np.float16)
    v_aug_all[..., :HN] = v5
    v_aug_all[..., HN] = 1.0
    v_aug_all = v_aug_all.reshape(B, NP, P, N_SK_TILES * (HN + 1))

    # mask-matmul constants: sc[m, n] += sum_p triT[p, m] * ident2[p, n]
    #   = triT[n%128, m]  which must be NEG where (n%128) < m
    triT = np.where(
        np.arange(P)[:, None] < np.arange(P)[None, :], NEG, 0.0
    ).astype(np.float16)                            # triT[p, c] = NEG if p < c
    ident2 = np.concatenate([np.eye(P), np.eye(P)], axis=1).astype(np.float16)

    in_maps = []
    for c in range(N_CORES):
        slots = _core_slots(c)
        im = {
            "qT": np.ascontiguousarray(np.stack([qT_all[b, n] for b, n in slots])),
            "kT": np.ascontiguousarray(np.stack([kT_all[b, n] for b, n in slots])),
            "v_aug": np.ascontiguousarray(
                np.stack([v_aug_all[b, n] for b, n in slots])
            ),
            "triT": triT,
            "ident2": ident2,
        }
        in_maps.append(im)
    return nc, in_maps


def assemble(results):
    """Gather per-core 'out' arrays into the full [SQ, B, NP*HN] output."""
    full = np.empty((SQ, B, NP * HN), dtype=np.float32)
    for c in range(N_CORES):
        o = results[c]["out"]  # [4, 4, 128, 512]
        for s, (b, n) in enumerate(_core_slots(c)):
            ctx = (
                o[s].reshape(4, P, 4, HN).transpose(0, 2, 1, 3).reshape(SQ, HN)
            )
            full[:, b, n * HN : (n + 1) * HN] = ctx
    return full


def kernel(query_layer, key_layer, value_layer, attention_mask):
    from concourse.bass_utils import run_bass_kernel_spmd

    nc, in_maps = prepare(query_layer, key_layer, value_layer, attention_mask)
    res = run_bass_kernel_spmd(nc, in_maps, list(range(N_CORES)))
    return assemble(res.results)


# revision 42
# speedup vs baseline: 1.2587x; 1.0185x over previous
"""Causal multi-head attention forward on 8 Trainium2 NeuronCores.

Problem: nn_CoreAttention (SQ=SK=2048, B=2, NP=16 heads, HN=128, fp32).

Sharding: 32 (batch, head) pairs split 4 per core (tensor-parallel over
heads, data-parallel over batch). No collectives.

Per (b, n) pair, in transposed score orientation (sk on partitions):
    scoresT[sk, sq] = (K Q^T)                 (PE matmul, fp16 in, hn contracted)
    expT = exp(scoresT * 1/sqrt(HN) + mask)   (ScalarE, fp16 out)
    ctx_aug[sq, hn+1] = expT^T @ [V | 1]      (PE matmul, sk contracted;
                                               col hn holds the softmax denom)
    ctx = ctx_aug[:, :hn] * 1/ctx_aug[:, hn]  (DVE reciprocal + scale)

v2 structure (vs the v1 baseline):
  - q/k cast to fp16 on host; no on-device casts; FWL stays enabled.
  - sq chunks of 256 with the two 128-tiles SWAPPED (host pre-swap), so the
    diagonal j-tile's masked half is the trailing half of the chunk: the
    diagonal QK matmul streams only 128 valid cols and exp skips the rest.
  - the two triangular tiles per chunk land on 256 contiguous score cols:
    one DVE add with a single [128,256] additive tile handles all masking.
  - PV is software-pipelined one chunk behind QK so the PE never stalls on
    the exp latency of the chunk it just produced.
  - slot 0 loads q/k/v in small pieces (fast start); the last slot walks
    chunks descending so the kernel tail is the smallest chunk.
  - outputs accumulate in SBUF and leave as 4 wide DMAs per slot.
"""

import math
import numpy as np
from contextlib import ExitStack

import concourse.bacc as bacc
import concourse.tile as tile
from concourse import mybir

SQ, SK, B, NP, HN = 2048, 2048, 2, 16, 128
N_CORES = 8
SLOTS_PER_CORE = 4
P = 128
CHUNK = 256
N_CHUNKS = SQ // CHUNK      # 8
N_SK_TILES = SK // P        # 16
NEG = -60000.0

import os
GROUP = int(os.environ.get("ATT_GROUP", "6"))          # j-tiles per PSUM group
SC_BUFS = int(os.environ.get("ATT_SC_BUFS", "2"))
CX_BUFS = int(os.environ.get("ATT_CX_BUFS", "2"))
E_BUFS = int(os.environ.get("ATT_E_BUFS", "8"))

F32 = mybir.dt.float32
F16 = mybir.dt.float16


def _build_program():
    nc = bacc.Bacc()

    qT_d = nc.declare_dram_parameter("qT", [SLOTS_PER_CORE, P, SQ], F16, isOutput=False)
    kT_d = nc.declare_dram_parameter("kT", [SLOTS_PER_CORE, P, SK], F16, isOutput=False)
    v_d = nc.declare_dram_parameter(
        "v_aug", [SLOTS_PER_CORE, P, N_SK_TILES * (HN + 1)], F16, isOutput=False
    )
    # triT[p, c] = NEG if p < c else 0; ident2 = [I | I]
    triT_d = nc.declare_dram_parameter("triT", [P, P], F16, isOutput=False)
    id2_d = nc.declare_dram_parameter("ident2", [P, 2 * P], F16, isOutput=False)
    out_d = nc.declare_dram_parameter(
        "out", [SLOTS_PER_CORE, 4, P, 4 * HN], F32, isOutput=True
    )

    inv_norm = 1.0 / math.sqrt(HN)

    with tile.TileContext(nc) as tc, ExitStack() as ctx:
        qk_pool = ctx.enter_context(tc.tile_pool(name="qk", bufs=2))
        v_pool = ctx.enter_context(tc.tile_pool(name="v", bufs=2))
        m_pool = ctx.enter_context(tc.tile_pool(name="m", bufs=1))
        e_pool = ctx.enter_context(tc.tile_pool(name="e", bufs=E_BUFS))
        o_pool = ctx.enter_context(tc.tile_pool(name="o", bufs=4))
        r_pool = ctx.enter_context(tc.tile_pool(name="r", bufs=4))
        sc_ps = ctx.enter_context(tc.tile_pool(name="sc", bufs=SC_BUFS, space="PSUM"))
        cx_ps = ctx.enter_context(tc.tile_pool(name="cx", bufs=CX_BUFS, space="PSUM"))

        triT_sb = m_pool.tile([P, P], F16, tag="triT")
        id2_sb = m_pool.tile([P, 2 * P], F16, tag="id2")

        # touch Exp immediately so the ACT table loads during the initial DMAs
        warm_in = m_pool.tile([P, 1], F32, tag="warm_in")
        nc.vector.memset(warm_in[:], 0.0)
        warm_out = m_pool.tile([P, 1], F32, tag="warm_out")
        nc.scalar.activation(
            warm_out[:], warm_in[:], mybir.ActivationFunctionType.Exp
        )



        # ---- per-slot input loading -------------------------------------
        def load_slot(slot):
            """Returns (kslice, qchunk, vslice) accessor fns for this slot."""
            if slot == 0:
                # pieces, issued in first-use order (chunks run descending)
                # across both HWDGE queues (sync + scalar run in parallel and
                # each queue moves ~55 GB/s, so split k into 4 transfers)
                # k piece boundaries (in j tiles): tiny first piece so the
                # very first QK matmuls + exp can fire ~1.5us after issue
                KB = (0, 2, 6, 12, 16)
                kts = [
                    qk_pool.tile(
                        [P, (KB[pc + 1] - KB[pc]) * P], F16,
                        tag=f"k{pc}", name=f"k{pc}",
                    )
                    for pc in range(4)
                ]
                qts = [
                    qk_pool.tile([P, CHUNK], F16, tag=f"q{pc}", name=f"q{pc}")
                    for pc in range(N_CHUNKS)
                ]
                vts = [
                    v_pool.tile([P, 8 * (HN + 1)], F16, tag=f"v{pc}", name=f"v{pc}")
                    for pc in range(2)
                ]
                nc.sync.dma_start(kts[0][:], kT_d[slot][:, 0 : KB[1] * P])
                nc.scalar.dma_start(qts[7][:], qT_d[slot][:, 7 * CHUNK : 8 * CHUNK])
                nc.sync.dma_start(kts[1][:], kT_d[slot][:, KB[1] * P : KB[2] * P])
                nc.scalar.dma_start(triT_sb[:], triT_d[:])
                nc.sync.dma_start(kts[2][:], kT_d[slot][:, KB[2] * P : KB[3] * P])
                nc.scalar.dma_start(kts[3][:], kT_d[slot][:, KB[3] * P : KB[4] * P])
                nc.sync.dma_start(id2_sb[:], id2_d[:])
                nc.sync.dma_start(qts[6][:], qT_d[slot][:, 6 * CHUNK : 7 * CHUNK])
                for pc in range(2):
                    nc.scalar.dma_start(
                        vts[pc][:],
                        v_d[slot][:, pc * 8 * (HN + 1) : (pc + 1) * 8 * (HN + 1)],
                    )
                for pc in range(N_CHUNKS - 3, -1, -1):
                    nc.sync.dma_start(
                        qts[pc][:], qT_d[slot][:, pc * CHUNK : (pc + 1) * CHUNK]
                    )

                def kslice(j):
                    for pc in range(4):
                        if j < KB[pc + 1]:
                            return kts[pc][:, (j - KB[pc]) * P : (j - KB[pc] + 1) * P]
                qchunk = lambda ci: qts[ci][:]
                vslice = lambda j: vts[j // 8][
                    :, (j % 8) * (HN + 1) : (j % 8 + 1) * (HN + 1)
                ]
            else:
                kt = qk_pool.tile([P, SK], F16, tag="k")
                nc.sync.dma_start(kt[:], kT_d[slot])
                qt = qk_pool.tile([P, SQ], F16, tag="q")
                nc.sync.dma_start(qt[:], qT_d[slot])
                vt = v_pool.tile([P, N_SK_TILES * (HN + 1)], F16, tag="v")
                nc.sync.dma_start(vt[:], v_d[slot])
                kslice = lambda j: kt[:, j * P : (j + 1) * P]
                qchunk = lambda ci: qt[:, ci * CHUNK : (ci + 1) * CHUNK]
                vslice = lambda j: vt[:, j * (HN + 1) : (j + 1) * (HN + 1)]
            return kslice, qchunk, vslice

        # ---- score packer (within a slot): QK blocks stream into shared
        # 1536-col PSUM tiles so most exp activations run at max width;
        # flushed at slot boundaries to avoid cross-slot coupling
        CAP = GROUP * CHUNK
        etmap = {}
        packer = {"sc": None, "fill": 0, "entries": []}

        def flush_packer():
            if packer["sc"] is None or packer["fill"] == 0:
                return
            et = e_pool.tile([P, CAP], F16, tag="expT", name="et")
            nc.scalar.activation(
                et[:, : packer["fill"]], packer["sc"][:, : packer["fill"]],
                mybir.ActivationFunctionType.Exp,
                scale=inv_norm,
            )
            for key, off in packer["entries"]:
                etmap[key] = (et, off)
            packer["sc"] = None
            packer["fill"] = 0
            packer["entries"] = []

        def emit_qk(slot, slot_io, ci):
            kslice, qchunk, _ = slot_io
            diag = 2 * ci + 1
            for j in range(2 * ci + 2):      # ascending; diagonal j last
                w = P if j == diag else CHUNK
                if packer["sc"] is None or packer["fill"] + w > CAP:
                    flush_packer()
                if packer["sc"] is None:
                    packer["sc"] = sc_ps.tile(
                        [P, CAP], F32, tag="scores", name="sc"
                    )
                sc, co = packer["sc"], packer["fill"]
                nc.tensor.matmul(
                    sc[:, co : co + w], kslice(j), qchunk(ci)[:, 0:w],
                    start=True, stop=True,
                )
                # causal mask on the PE: sc[m, n] += triT[n%128, m].
                # Must directly follow its QK matmul — start=False
                # continues only the most recent accumulation group.
                if j == diag:
                    nc.tensor.matmul(
                        sc[:, co : co + P], triT_sb[:], id2_sb[:, 0:P],
                        start=False, stop=True,
                    )
                elif j == diag - 1:
                    nc.tensor.matmul(
                        sc[:, co + P : co + 2 * P], triT_sb[:], id2_sb[:, 0:P],
                        start=False, stop=True,
                    )
                packer["entries"].append(((slot, ci, j), co))
                packer["fill"] = co + w
                # ramp: flush small tiles early in the very first chunk so
                # the exp stream starts as soon as the first k piece lands
                if slot == 0 and ci == N_CHUNKS - 1 and j in (1, 5):
                    flush_packer()

        # ---- emit one chunk's PV + normalize + (maybe) out DMA ----------
        def emit_pv(slot, slot_io, ci, oq_tiles, done_quarters):
            _, _, vslice = slot_io
            exp_tiles = {j: etmap[(slot, ci, j)] for j in range(2 * ci + 2)}
            # one PSUM tile holds both context vectors of the chunk:
            # i_lo at cols [0,129), i_hi at cols [129,258)
            cx = cx_ps.tile([P, 2 * (HN + 1)], F32, tag="ctx")
            for i in (2 * ci + 1, 2 * ci):   # i_hi (first half of chunk), i_lo
                off = 0 if i == 2 * ci + 1 else P
                base = (HN + 1) if i == 2 * ci + 1 else 0
                pv_js = list(range(i + 1))
                for idx, j in enumerate(pv_js):
                    et, co = exp_tiles[j]
                    nc.tensor.matmul(
                        cx[:, base : base + HN + 1],
                        et[:, co + off : co + off + P], vslice(j),
                        start=(idx == 0), stop=(idx == len(pv_js) - 1),
                    )
            recip = r_pool.tile([P, 2], F32, tag="recip")
            nc.vector.reciprocal(
                recip[:], cx[:, HN : 2 * HN + 2 : HN + 1]
            )
            qt_idx = (2 * ci) // 4
            if qt_idx not in oq_tiles:
                oq_tiles[qt_idx] = o_pool.tile(
                    [P, 4 * HN], F32, tag="oq", name="oq"
                )
            ot = oq_tiles[qt_idx]
            col = (2 * ci % 4) * HN          # i_lo column; i_hi is the next one
            nc.vector.tensor_mul(
                ot[:, col : col + 2 * HN].rearrange("p (s c) -> p s c", s=2),
                cx[:].rearrange("p (s c) -> p s c", s=2)[:, :, 0:HN],
                recip[:].rearrange("p (s c) -> p s c", c=1).broadcast_to(
                    [P, 2, HN]
                ),
            )
            if slot == SLOTS_PER_CORE - 1:
                # last slot: ship each chunk's half-quarter as soon as it is
                # normalized, so the final DMA (and the epilogue's DMA drain)
                # starts as early as possible
                h = ci % 2
                nc.sync.dma_start(
                    out_d[slot, qt_idx][:, h * 2 * HN : (h + 1) * 2 * HN],
                    ot[:, h * 2 * HN : (h + 1) * 2 * HN],
                )
            else:
                done_quarters.setdefault(qt_idx, set()).add(ci)
                if len(done_quarters[qt_idx]) == 2:
                    nc.sync.dma_start(out_d[slot, qt_idx], oq_tiles[qt_idx][:])

        # ---- main schedule: PV runs as soon as its exp tiles exist ------
        pvq = []  # [(slot, slot_io, ci, oq_tiles, done_quarters)]

        def drain_pv(final=False):
            # keep one chunk pending (unless final) so PV trails the QK
            # stream and the PE never queues behind a just-issued exp;
            # a chunk is ready once its diagonal block has been exp'd
            while pvq and (final or len(pvq) >= 2):
                slot, slot_io, ci, oq, dq = pvq[0]
                if (slot, ci, 2 * ci + 1) not in etmap:
                    return
                pvq.pop(0)
                emit_pv(slot, slot_io, ci, oq, dq)

        slot_state = {}
        for slot in range(SLOTS_PER_CORE):
            slot_io = load_slot(slot)
            slot_state[slot] = ({}, {})  # oq_tiles, done_quarters
            for ci in range(N_CHUNKS - 1, -1, -1):
                emit_qk(slot, slot_io, ci)
                oq, dq = slot_state[slot]
                pvq.append((slot, slot_io, ci, oq, dq))
                drain_pv()
            flush_packer()   # keep exp tiles slot-local
            drain_pv()
        drain_pv(final=True)
        assert not pvq

    nc.compile()
    return nc


_cache = {}


def _get_program(mask: np.ndarray):
    # this kernel is specialized to the standard causal mask
    m = np.asarray(mask)
    causal = np.triu(np.ones((SQ, SK), dtype=bool), k=1)
    for b in range(B):
        if not np.array_equal(m[b, 0], causal):
            raise ValueError("kernel specialized to causal attention mask")
    if "nc" not in _cache:
        _cache["nc"] = _build_program()
    return _cache["nc"]


def _core_slots(c):
    return [(0, 2 * c), (0, 2 * c + 1), (1, 2 * c), (1, 2 * c + 1)]


def prepare(query_layer, key_layer, value_layer, attention_mask):
    q = np.asarray(query_layer)
    k = np.asarray(key_layer)
    v = np.asarray(value_layer)
    nc = _get_program(np.asarray(attention_mask))

    # qT with the two 128-col tiles of each 256 chunk swapped:
    # sbuf layout col (256*ci + [0..255]) = sq (256*ci + [128..255, 0..127])
    q16 = q.astype(np.float16)                      # [SQ, B, NP, HN]
    qv = q16.reshape(N_CHUNKS, 2, P, B, NP, HN)[:, ::-1]   # swap tile pairs
    qT_all = np.ascontiguousarray(qv.transpose(3, 4, 5, 0, 1, 2)).reshape(
        B, NP, HN, SQ
    )
    k16 = k.astype(np.float16)
    kT_all = np.ascontiguousarray(k16.transpose(1, 2, 3, 0))  # [B, NP, HN, SK]

    v5 = v.reshape(N_SK_TILES, P, B, NP, HN).transpose(2, 3, 1, 0, 4)
    v_aug_all = np.empty((B, NP, P, N_SK_TILES, HN + 1), dtype=np.float16)
    v_aug_all[..., :HN] = v5
    v_aug_all[..., HN] = 1.0
    v_aug_all = v_aug_all.reshape(B, NP, P, N_SK_TILES * (HN + 1))

    # mask-matmul constants: sc[m, n] += sum_p triT[p, m] * ident2[p, n]
    #   = triT[n%128, m]  which must be NEG where (n%128) < m
    triT = np.where(
        np.arange(P)[:, None] < np.arange(P)[None, :], NEG, 0.0
    ).astype(np.float16)                            # triT[p, c] = NEG if p < c
    ident2 = np.concatenate([np.eye(P), np.eye(P)], axis=1).astype(np.float16)

    in_maps = []
    for c in range(N_CORES):
        slots = _core_slots(c)
        im = {
            "qT": np.ascontiguousarray(np.stack([qT_all[b, n] for b, n in slots])),
            "kT": np.ascontiguousarray(np.stack([kT_all[b, n] for b, n in slots])),
            "v_aug": np.ascontiguousarray(
                np.stack([v_aug_all[b, n] for b, n in slots])
            ),
            "triT": triT,
            "ident2": ident2,
        }
        in_maps.append(im)
    return nc, in_maps


def assemble(results):
    """Gather per-core 'out' arrays into the full [SQ, B, NP*HN] output."""
    full = np.empty((SQ, B, NP * HN), dtype=np.float32)
    for c in range(N_CORES):
        o = results[c]["out"]  # [4, 4, 128, 512]
        for s, (b, n) in enumerate(_core_slots(c)):
            ctx = (
                o[s].reshape(4, P, 4, HN).transpose(0, 2, 1, 3).reshape(SQ, HN)
            )
            full[:, b, n * HN : (n + 1) * HN] = ctx
    return full


def kernel(query_layer, key_layer, value_layer, attention_mask):
    from concourse.bass_utils import run_bass_kernel_spmd

    nc, in_maps = prepare(query_layer, key_layer, value_layer, attention_mask)
    res = run_bass_kernel_spmd(nc, in_maps, list(range(N_CORES)))
    return assemble(res.results)
